# revision 6
# baseline (speedup 1.0000x reference)
"""Causal self-attention block (qkv proj -> causal MHA -> out proj) on 8 TRN2
NeuronCores.

Sharding: core c -> (batch b = c//2, head-group g = c%2). Each core computes
its batch's attention for 8 of the 16 heads (Megatron column-parallel qkv,
row-parallel out-proj), then an AllReduce over the 2-core group of each batch.

Layout choices (all host-side permutations are done in numpy here):
  - Q^T/K^T are produced directly in [c', t] orientation (c' on partitions) by
    using w_qkv chunks as the stationary matmul operand and x^T as the moving
    one; x^T comes from PE-transposes of x tiles.
  - Head pairs are packed into 128-partition tiles (head 2p in partitions
    0:64, head 2p+1 in 64:128) so the QK^T matmuls of both heads run
    concurrently in the PE array via row tiling (tile_position).
  - V keeps the [t, c'] orientation with an extra all-ones column per head, so
    the A@V matmul (M=65) also produces the softmax row-sums for free.
  - Normalization (1/rowsum) is broadcast across partitions with a K=1 matmul
    and applied with one DVE multiply; the out-proj consumes Y^T directly as
    its stationary operand, yielding outputs in natural [t, c] orientation.
  - All matmuls run as float32r (full PE rate at N>=512 moving dim).
"""

import os
import numpy as np

import concourse.bass as bass
import concourse.bacc as bacc
import concourse.mybir as mybir
import concourse.tile as tile
from concourse.bass_utils import run_bass_kernel_spmd
from concourse.masks import make_identity
from contextlib import ExitStack

F32 = mybir.dt.float32
BF16 = mybir.dt.bfloat16
F32R = mybir.dt.float32r
P = 128
D = 64


def build_program(T, C, HC, num_devices, groups, qk_dt=BF16):
    """Build the per-core SPMD program.

    T: sequence length, C: model dim, HC: heads on this core (pair-packed),
    groups: all-reduce replica groups.
    """
    NPAIR = HC // 2
    CT = C // P            # contraction chunks of the qkv matmul
    NT5 = T // 512         # 512-wide t tiles
    NTS = T // P           # 128-wide t subtiles
    CSH = HC * D           # local c-tilde width (V / Y^T columns)
    NCO = C // 512         # 512-wide out-column tiles
    QKW = 2 * HC * D       # q+k c' columns
    NQK = QKW // P         # qk c'-tiles (2 per head pair)
    SCALE = 1.0 / np.sqrt(np.float32(D))
    NEG = -1.0e9

    nc = bacc.Bacc("TRN2", target_bir_lowering=False, debug=False,
                   num_devices=num_devices)
    xb = nc.dram_tensor("xb", [T, C], F32, kind="ExternalInput").ap()
    wqk = nc.dram_tensor("wqk", [C, QKW], F32R, kind="ExternalInput").ap()
    wv = nc.dram_tensor("wv", [C, CSH], F32R, kind="ExternalInput").ap()
    bqk = nc.dram_tensor("bqk", [P, NQK], F32, kind="ExternalInput").ap()
    bv = nc.dram_tensor("bv", [1, CSH], F32R, kind="ExternalInput").ap()
    bpr = nc.dram_tensor("bpr", [1, C], F32R, kind="ExternalInput").ap()
    wpr = nc.dram_tensor("wpr", [CSH, C], F32R, kind="ExternalInput").ap()
    onespp = nc.dram_tensor("onespp", [P, P], F32R, kind="ExternalInput").ap()
    out = nc.dram_tensor("out", [T, C], F32, kind="ExternalOutput").ap()

    ADD = mybir.AluOpType.add
    MULT = mybir.AluOpType.mult
    EXP = mybir.ActivationFunctionType.Exp

    with tile.TileContext(nc) as tc, ExitStack() as ctx:
        # ---- persistent pools ------------------------------------------
        const = ctx.enter_context(tc.tile_pool(name="const", bufs=1))
        ident = const.tile([P, P], F32, name="ident")
        make_identity(nc, ident)
        ones = const.tile([1, P], F32R, name="ones")
        nc.sync.dma_start(out=ones[:], in_=onespp[0:1, :])
        bqk_sb = const.tile([P, NQK], F32, name="bqk_sb")
        nc.sync.dma_start(out=bqk_sb[:], in_=bqk[:])
        bv_sb = const.tile([1, CSH], F32R, name="bv_sb")
        nc.sync.dma_start(out=bv_sb[:], in_=bv[:])
        bpr_sb = const.tile([1, C], F32R, name="bpr_sb")
        nc.sync.dma_start(out=bpr_sb[:], in_=bpr[:])

        qk_pool = ctx.enter_context(tc.tile_pool(name="qkp", bufs=1))
        QK_sb = [qk_pool.tile([P, T], qk_dt, name=f"qk{ct}") for ct in range(NQK)]
        v_pool = ctx.enter_context(tc.tile_pool(name="vp", bufs=1))
        V_sb = [v_pool.tile([P, HC * 65], F32R, name=f"v{i}") for i in range(NTS)]
        yn_pool = ctx.enter_context(tc.tile_pool(name="ynp", bufs=1))
        Yn_sb = [yn_pool.tile([P, T], F32R, name=f"yn{p}") for p in range(NPAIR)]
        dram = ctx.enter_context(tc.tile_pool(name="dram", bufs=1, space="DRAM"))
        ob = dram.tile([T, C], F32, name="ob")
        # pairwise groups: Shared-scratchpad outputs only exist for >4-core
        # groups, so the all-reduce output stays Local here
        orr = dram.tile([T, C], F32, name="orr")

        # ---- stage A: x^T and qkv projections --------------------------
        with ExitStack() as actx:
            wqk_pool = actx.enter_context(tc.tile_pool(name="wqkp", bufs=1))
            wqk_sb = [wqk_pool.tile([P, QKW], F32R, name=f"wqk{j}") for j in range(CT)]
            for j in range(CT):
                nc.sync.dma_start(out=wqk_sb[j][:], in_=wqk[j * P:(j + 1) * P, :])
            wv_pool = actx.enter_context(tc.tile_pool(name="wvp", bufs=1))
            wv_sb = [wv_pool.tile([P, CSH], F32R, name=f"wv{j}") for j in range(CT)]
            for j in range(CT):
                nc.sync.dma_start(out=wv_sb[j][:], in_=wv[j * P:(j + 1) * P, :])

            xr_pool = actx.enter_context(tc.tile_pool(name="xrp", bufs=4))
            xt_pool = actx.enter_context(tc.tile_pool(name="xtp", bufs=CT))
            psx = actx.enter_context(tc.tile_pool(name="psx", bufs=2, space="PSUM"))
            psqk = actx.enter_context(tc.tile_pool(name="psqk", bufs=2, space="PSUM"))
            psv = actx.enter_context(tc.tile_pool(name="psv", bufs=2, space="PSUM"))

            for tt in range(NT5):
                xrs = []
                for i in range(4):
                    xr = xr_pool.tile([P, C], F32, name="xr")
                    nc.sync.dma_start(
                        out=xr[:], in_=xb[(tt * 4 + i) * P:(tt * 4 + i + 1) * P, :])
                    xrs.append(xr)
                xts = []
                for j in range(CT):
                    ps = psx.tile([P, 512], F32, name="psx")
                    for i in range(4):
                        nc.tensor.transpose(
                            ps[:, i * P:(i + 1) * P],
                            xrs[i][:, j * P:(j + 1) * P], ident[:])
                    xt = xt_pool.tile([P, 512], F32R, name="xt")
                    nc.scalar.copy(xt[:], ps[:])
                    xts.append(xt)
                # Q^T / K^T tiles (c' on partitions), pair-packed
                for ct in range(NQK):
                    ps = psqk.tile([P, 512], F32, name="psqk")
                    for j in range(CT):
                        nc.tensor.matmul(
                            ps[:], wqk_sb[j][:, ct * P:(ct + 1) * P],
                            xts[j][:], start=(j == 0), stop=(j == CT - 1))
                    nc.vector.tensor_scalar_add(
                        QK_sb[ct][:, tt * 512:(tt + 1) * 512], ps[:],
                        bqk_sb[:, ct:ct + 1])
                # V tiles ([t, c'] orientation) + ones column per head
                for i in range(4):
                    ps = psv.tile([P, CSH], F32, name="psv")
                    for j in range(CT):
                        nc.tensor.matmul(
                            ps[:], xts[j][:, i * P:(i + 1) * P],
                            wv_sb[j][:], start=(j == 0), stop=False)
                    nc.tensor.matmul(ps[:], ones[:], bv_sb[:],
                                     start=False, stop=True)
                    vt = V_sb[tt * 4 + i]
                    vt3 = vt.rearrange("p (h e) -> p h e", e=65)
                    nc.scalar.copy(
                        vt3[:, :, 0:64], ps.rearrange("p (h d) -> p h d", d=D))
                    nc.sync.dma_start(out=vt3[:, :, 64:65], in_=onespp[0:P, 0:HC])

        # ---- stage B: causal attention per head pair -------------------
        with ExitStack() as bctx:
            mpool = bctx.enter_context(tc.tile_pool(name="mpool", bufs=1))
            masks = []
            for i in range(4):
                mk = mpool.tile([P, 512], F32, name=f"mask{i}")
                nc.gpsimd.memset(mk[:], 0.0)
                nc.gpsimd.affine_select(
                    out=mk[:], in_=mk[:], compare_op=mybir.AluOpType.is_ge,
                    fill=NEG, base=-(P * i), pattern=[[1, 512]],
                    channel_multiplier=-1)
                masks.append(mk)

            st_pool = bctx.enter_context(tc.tile_pool(name="stp", bufs=2, space="PSUM"))
            psY = bctx.enter_context(tc.tile_pool(name="psY", bufs=1, space="PSUM"))
            bcp = bctx.enter_context(tc.tile_pool(name="bcp", bufs=1, space="PSUM"))
            exp_pool = bctx.enter_context(tc.tile_pool(name="expp", bufs=2))
            yc_pool = bctx.enter_context(tc.tile_pool(name="ycp", bufs=2))
            r_pool = bctx.enter_context(tc.tile_pool(name="rp", bufs=2))
            ynb_pool = bctx.enter_context(tc.tile_pool(name="ynbp", bufs=2))

            for p in range(NPAIR):
                qa = QK_sb[2 * p]
                ka = QK_sb[2 * p + 1]
                hA, hB = 2 * p, 2 * p + 1
                for qt in range(NT5):
                    nkt = 4 * qt + 4
                    qs = slice(qt * 512, (qt + 1) * 512)
                    pyA = psY.tile([65, 512], F32, name="pyA")
                    pyB = psY.tile([65, 512], F32, name="pyB")
                    pend = []  # software-pipeline AV one step behind QK on PE

                    def emit_av(kt, eA, eB, nkt=nkt, pyA=pyA, pyB=pyB,
                                hA=hA, hB=hB):
                        vt = V_sb[kt]
                        nc.tensor.matmul(
                            pyA[:], vt[:, hA * 65:(hA + 1) * 65], eA[:],
                            start=(kt == 0), stop=(kt == nkt - 1))
                        nc.tensor.matmul(
                            pyB[:], vt[:, hB * 65:(hB + 1) * 65], eB[:],
                            start=(kt == 0), stop=(kt == nkt - 1))

                    for kt in range(nkt):
                        stA = st_pool.tile([P, 512], F32, name="stA")
                        stB = st_pool.tile([P, 512], F32, name="stB")
                        nc.tensor.matmul(
                            stA[:], ka[0:64, kt * P:(kt + 1) * P],
                            qa[0:64, qs])
                        nc.tensor.matmul(
                            stB[:], ka[64:128, kt * P:(kt + 1) * P],
                            qa[64:128, qs], tile_position=(64, 0))
                        di = kt - 4 * qt
                        if di >= 0:
                            nc.vector.tensor_tensor(stA[:], stA[:], masks[di][:], op=ADD)
                            nc.vector.tensor_tensor(stB[:], stB[:], masks[di][:], op=ADD)
                        eA = exp_pool.tile([P, 512], F32R, name="eA")
                        nc.scalar.activation(eA[:], stA[:], EXP, scale=SCALE)
                        eB = exp_pool.tile([P, 512], F32R, name="eB")
                        nc.scalar.activation(eB[:], stB[:], EXP, scale=SCALE)
                        pend.append((kt, eA, eB))
                        if len(pend) > 1:
                            emit_av(*pend.pop(0))
                    emit_av(*pend.pop(0))

                    # normalization
                    rA = r_pool.tile([1, 512], F32R, name="rA")
                    rB = r_pool.tile([1, 512], F32R, name="rB")
                    with nc.allow_low_precision(reason="softmax denom is f32r"):
                        nc.vector.reciprocal(rA[:], pyA[64:65, :])
                        nc.vector.reciprocal(rB[:], pyB[64:65, :])
                    bcA = bcp.tile([64, 512], F32, name="bcA")
                    nc.tensor.matmul(bcA[:], ones[:, 0:64], rA[:])
                    bcB = bcp.tile([64, 512], F32, name="bcB")
                    nc.tensor.matmul(bcB[:], ones[:, 0:64], rB[:])
                    ycA = yc_pool.tile([64, 512], F32, name="ycA")
                    nc.scalar.copy(ycA[:], pyA[0:64, :])
                    ycB = yc_pool.tile([64, 512], F32, name="ycB")
                    nc.scalar.copy(ycB[:], pyB[0:64, :])
                    nc.vector.tensor_tensor(
                        Yn_sb[p][0:64, qs], ycA[:], bcA[:], op=MULT)
                    ynB = ynb_pool.tile([64, 512], F32R, name="ynB")
                    nc.vector.tensor_tensor(ynB[:], ycB[:], bcB[:], op=MULT)
                    nc.sync.dma_start(out=Yn_sb[p][64:128, qs], in_=ynB[:])

        # ---- stage C: out projection + pair AllReduce ------------------
        with ExitStack() as cctx:
            wpr_pool = cctx.enter_context(tc.tile_pool(name="wprp", bufs=1))
            wpr_sb = [wpr_pool.tile([P, C], F32R, name=f"wpr{p}") for p in range(NPAIR)]
            for p in range(NPAIR):
                nc.sync.dma_start(out=wpr_sb[p][:], in_=wpr[p * P:(p + 1) * P, :])
            psO = cctx.enter_context(tc.tile_pool(name="psO", bufs=2, space="PSUM"))
            oc_pool = cctx.enter_context(tc.tile_pool(name="ocp", bufs=3))

            for ts in range(NTS):
                for co in range(NCO):
                    po = psO.tile([P, 512], F32, name="po")
                    for p in range(NPAIR):
                        nc.tensor.matmul(
                            po[:], Yn_sb[p][:, ts * P:(ts + 1) * P],
                            wpr_sb[p][:, co * 512:(co + 1) * 512],
                            start=(p == 0), stop=False)
                    nc.tensor.matmul(po[:], ones[:],
                                     bpr_sb[:, co * 512:(co + 1) * 512],
                                     start=False, stop=True)
                    oc = oc_pool.tile([P, 512], F32, name="oc")
                    nc.scalar.copy(oc[:], po[:])
                    nc.sync.dma_start(
                        out=ob[ts * P:(ts + 1) * P, co * 512:(co + 1) * 512],
                        in_=oc[:])

            nc.gpsimd.collective_compute(
                "AllReduce", mybir.AluOpType.add, replica_groups=groups,
                ins=[ob.opt()], outs=[orr.opt()])
            for i in range(NT5):
                nc.sync.dma_start(out=out[i * 512:(i + 1) * 512, :],
                                  in_=orr[i * 512:(i + 1) * 512, :])

    nc.compile()
    return nc


def prep_core_inputs(x, w_qkv, b_qkv, w_proj, b_proj, b, g, HC):
    """Host-side shard + layout permutation for core (batch b, head group g)."""
    C = x.shape[-1]
    heads = [g * HC + i for i in range(HC)]
    wq, wk, wvf = w_qkv[:, 0:C], w_qkv[:, C:2 * C], w_qkv[:, 2 * C:3 * C]
    bq, bk, bvf = b_qkv[0:C], b_qkv[C:2 * C], b_qkv[2 * C:3 * C]
    qk_cols, bqk_cols = [], []
    for p in range(HC // 2):
        h0, h1 = heads[2 * p], heads[2 * p + 1]
        qk_cols += [wq[:, h0 * D:(h0 + 1) * D], wq[:, h1 * D:(h1 + 1) * D]]
        bqk_cols += [bq[h0 * D:(h0 + 1) * D], bq[h1 * D:(h1 + 1) * D]]
        qk_cols += [wk[:, h0 * D:(h0 + 1) * D], wk[:, h1 * D:(h1 + 1) * D]]
        bqk_cols += [bk[h0 * D:(h0 + 1) * D], bk[h1 * D:(h1 + 1) * D]]
    wqk = np.ascontiguousarray(np.concatenate(qk_cols, axis=1), np.float32)
    bqk = np.concatenate(bqk_cols).astype(np.float32)
    bqk = np.ascontiguousarray(bqk.reshape(-1, P).T)
    wv = np.ascontiguousarray(
        np.concatenate([wvf[:, h * D:(h + 1) * D] for h in heads], axis=1),
        np.float32)
    bv = np.ascontiguousarray(
        np.concatenate([bvf[h * D:(h + 1) * D] for h in heads])[None, :],
        np.float32)
    wpr = np.ascontiguousarray(
        np.concatenate([w_proj[h * D:(h + 1) * D, :] for h in heads], axis=0),
        np.float32)
    bpr = np.ascontiguousarray((b_proj / 2.0)[None, :], np.float32)
    return {
        "xb": np.ascontiguousarray(x[b], np.float32),
        "wqk": wqk, "bqk": bqk, "wv": wv, "bv": bv,
        "wpr": wpr, "bpr": bpr,
        "onespp": np.ones((128, 128), np.float32),
    }


_CACHE = {}


def kernel(x, w_qkv, b_qkv, w_proj, b_proj, _trace=False):
    x = np.asarray(x, np.float32)
    w_qkv = np.asarray(w_qkv, np.float32)
    b_qkv = np.asarray(b_qkv, np.float32)
    w_proj = np.asarray(w_proj, np.float32)
    b_proj = np.asarray(b_proj, np.float32)
    B, T, C = x.shape
    H = 16
    HC = H // 2
    groups = [[2 * b, 2 * b + 1] for b in range(B)]

    key = (T, C, HC, 2 * B)
    if key not in _CACHE:
        _CACHE[key] = build_program(T, C, HC, 2 * B, groups)
    nc = _CACHE[key]

    in_maps = []
    for c in range(2 * B):
        in_maps.append(
            prep_core_inputs(x, w_qkv, b_qkv, w_proj, b_proj, c // 2, c % 2, HC))
    res = run_bass_kernel_spmd(nc, in_maps, list(range(2 * B)), trace=_trace)
    out = np.stack([res.results[2 * b]["out"] for b in range(B)], axis=0)
    if _trace:
        return out, res
    return out


# revision 10
# speedup vs baseline: 1.5522x; 1.5522x over previous
"""Causal self-attention block (qkv proj -> causal MHA -> out proj) on 8 TRN2
NeuronCores.

Sharding: core c -> (batch b = c//2, head-group g = c%2). Each core computes
its batch's attention for 8 of the 16 heads (Megatron column-parallel qkv,
row-parallel out-proj), then a chunked pairwise ReduceScatter; each core
returns one half of its batch's rows and the host reassembles.

Layout / algorithm notes:
  - Q^T/K^T are produced directly in [c', t] orientation (c' on partitions) by
    using w_qkv chunks as the stationary matmul operand and x^T as the moving
    one; x^T comes from PE-transposes of x tiles.
  - Head pairs are packed into 128-partition tiles (head 2p in partitions
    0:64, head 2p+1 in 64:128) so the QK^T matmuls of both heads run
    concurrently in the PE array via row tiling (tile_position), writing the
    two halves of one 2-bank PSUM tile; causal mask-add and exp then each run
    as a single instruction over both heads.
  - Diagonal blocks are column-trimmed: only the causally-visible column
    range is computed through QK^T, mask, exp and A@V.
  - V keeps the [t, c'] orientation with an extra all-ones column per head, so
    the A@V matmul (M=65) also produces the softmax row-sums for free.
  - Normalization: fast approximate reciprocal of the row-sums, broadcast
    across partitions with a K=1 fp32 matmul, one DVE multiply per head.
  - Matmul operands are bf16 (accumulation stays fp32 in PSUM); the fp32
    x-transposes and psum evictions feed rounded-bf16 tiles.
"""

import numpy as np
import ml_dtypes

import concourse.bass as bass
import concourse.bacc as bacc
import concourse.mybir as mybir
import concourse.tile as tile
from concourse.bass_utils import run_bass_kernel_spmd
from concourse.masks import make_identity
from contextlib import ExitStack

F32 = mybir.dt.float32
BF16 = mybir.dt.bfloat16
P = 128
D = 64
BF16NP = ml_dtypes.bfloat16


def build_program(T, C, HC, num_devices, groups):
    NPAIR = HC // 2
    CT = C // P            # contraction chunks of the qkv matmul
    NT5 = T // 512         # 512-wide t tiles
    NTS = T // P           # 128-wide t subtiles
    CSH = HC * D           # local c-tilde width (V / Y^T columns)
    NCO = C // 512         # 512-wide out-column tiles
    QKW = 2 * HC * D       # q+k c' columns
    NQK = QKW // P         # qk c'-tiles (2 per head pair)
    SCALE = 1.0 / np.sqrt(np.float32(D))
    NEG = -1.0e9

    nc = bacc.Bacc("TRN2", target_bir_lowering=False, debug=False,
                   num_devices=num_devices)
    xb = nc.dram_tensor("xb", [T, C], F32, kind="ExternalInput").ap()
    wqk = nc.dram_tensor("wqk", [C, QKW], BF16, kind="ExternalInput").ap()
    wv = nc.dram_tensor("wv", [C, CSH], BF16, kind="ExternalInput").ap()
    bqk = nc.dram_tensor("bqk", [P, NQK], F32, kind="ExternalInput").ap()
    bvb = nc.dram_tensor("bvb", [P, CSH], F32, kind="ExternalInput").ap()
    bprb = nc.dram_tensor("bprb", [P, C], F32, kind="ExternalInput").ap()
    wpr = nc.dram_tensor("wpr", [CSH, C], BF16, kind="ExternalInput").ap()
    onespp = nc.dram_tensor("onespp", [P, P], BF16, kind="ExternalInput").ap()
    out = nc.dram_tensor("out", [T // 2, C], F32, kind="ExternalOutput").ap()

    ADD = mybir.AluOpType.add
    MULT = mybir.AluOpType.mult
    EXP = mybir.ActivationFunctionType.Exp

    with tile.TileContext(nc) as tc, ExitStack() as ctx:
        # ---- persistent pools ------------------------------------------
        const = ctx.enter_context(tc.tile_pool(name="const", bufs=1))
        ident = const.tile([P, P], F32, name="ident")
        make_identity(nc, ident)
        ones = const.tile([1, 64], F32, name="ones")
        nc.vector.memset(ones[:], 1.0)
        bqk_sb = const.tile([P, NQK], F32, name="bqk_sb")
        nc.sync.dma_start(out=bqk_sb[:], in_=bqk[:])
        bvb_sb = const.tile([P, CSH], F32, name="bvb_sb")
        nc.sync.dma_start(out=bvb_sb[:], in_=bvb[:])
        bprb_sb = const.tile([P, C], F32, name="bprb_sb")
        nc.sync.dma_start(out=bprb_sb[:], in_=bprb[:])

        qk_pool = ctx.enter_context(tc.tile_pool(name="qkp", bufs=1))
        QK_sb = [qk_pool.tile([P, T], BF16, name=f"qk{ct}") for ct in range(NQK)]
        v_pool = ctx.enter_context(tc.tile_pool(name="vp", bufs=1))
        V_sb = [v_pool.tile([P, HC * 65], BF16, name=f"v{i}") for i in range(NTS)]
        yn_pool = ctx.enter_context(tc.tile_pool(name="ynp", bufs=1))
        Yn_sb = [yn_pool.tile([P, T], BF16, name=f"yn{p}") for p in range(NPAIR)]
        dram = ctx.enter_context(tc.tile_pool(name="dram", bufs=1, space="DRAM"))
        ob = dram.tile([T, C], F32, name="ob")
        orh = [dram.tile([T // (2 * NT5), C], F32, name=f"orh{c}")
               for c in range(NT5)]

        # ---- stage A: x^T and qkv projections --------------------------
        with ExitStack() as actx:
            wqk_pool = actx.enter_context(tc.tile_pool(name="wqkp", bufs=1))
            wqk_sb = [wqk_pool.tile([P, QKW], BF16, name=f"wqk{j}") for j in range(CT)]
            for j in range(CT):
                nc.sync.dma_start(out=wqk_sb[j][:], in_=wqk[j * P:(j + 1) * P, :])
            wv_pool = actx.enter_context(tc.tile_pool(name="wvp", bufs=1))
            wv_sb = [wv_pool.tile([P, CSH], BF16, name=f"wv{j}") for j in range(CT)]
            for j in range(CT):
                nc.sync.dma_start(out=wv_sb[j][:], in_=wv[j * P:(j + 1) * P, :])

            xr_pool = actx.enter_context(tc.tile_pool(name="xrp", bufs=4))
            xt_pool = actx.enter_context(tc.tile_pool(name="xtp", bufs=CT))
            psx = actx.enter_context(tc.tile_pool(name="psx", bufs=2, space="PSUM"))
            psqk = actx.enter_context(tc.tile_pool(name="psqk", bufs=2, space="PSUM"))
            psv = actx.enter_context(tc.tile_pool(name="psv", bufs=2, space="PSUM"))

            for tt in range(NT5):
                xrs = []
                for i in range(4):
                    xr = xr_pool.tile([P, C], F32, name="xr")
                    nc.sync.dma_start(
                        out=xr[:], in_=xb[(tt * 4 + i) * P:(tt * 4 + i + 1) * P, :])
                    xrs.append(xr)
                xts = []
                for j in range(CT):
                    ps = psx.tile([P, 512], F32, name="psx")
                    for i in range(4):
                        nc.tensor.transpose(
                            ps[:, i * P:(i + 1) * P],
                            xrs[i][:, j * P:(j + 1) * P], ident[:])
                    xt = xt_pool.tile([P, 512], BF16, name="xt")
                    nc.scalar.copy(xt[:], ps[:])
                    xts.append(xt)
                # Q^T / K^T tiles (c' on partitions), pair-packed
                for ct in range(NQK):
                    ps = psqk.tile([P, 512], F32, name="psqk")
                    for j in range(CT):
                        nc.tensor.matmul(
                            ps[:], wqk_sb[j][:, ct * P:(ct + 1) * P],
                            xts[j][:], start=(j == 0), stop=(j == CT - 1))
                    nc.vector.tensor_scalar_add(
                        QK_sb[ct][:, tt * 512:(tt + 1) * 512], ps[:],
                        bqk_sb[:, ct:ct + 1])
                # V tiles ([t, c'] orientation) + ones column per head
                for i in range(4):
                    ps = psv.tile([P, CSH], F32, name="psv")
                    for j in range(CT):
                        nc.tensor.matmul(
                            ps[:], xts[j][:, i * P:(i + 1) * P],
                            wv_sb[j][:], start=(j == 0), stop=(j == CT - 1))
                    vt = V_sb[tt * 4 + i]
                    vt3 = vt.rearrange("p (h e) -> p h e", e=65)
                    nc.vector.tensor_tensor(
                        vt3[:, :, 0:64], ps.rearrange("p (h d) -> p h d", d=D),
                        bvb_sb.rearrange("p (h d) -> p h d", d=D), op=ADD)
                    nc.vector.memset(vt3[:, :, 64:65], 1.0)

        # ---- stage B: causal attention per head pair -------------------
        with ExitStack() as bctx:
            mpool = bctx.enter_context(tc.tile_pool(name="mpool", bufs=1))
            masks = []
            for i in range(4):
                mk = mpool.tile([P, 512], F32, name=f"mask{i}")
                nc.gpsimd.memset(mk[:], 0.0)
                nc.gpsimd.affine_select(
                    out=mk[:], in_=mk[:], compare_op=mybir.AluOpType.is_ge,
                    fill=NEG, base=-(P * i), pattern=[[1, 512]],
                    channel_multiplier=-1)
                masks.append(mk)

            st_pool = bctx.enter_context(tc.tile_pool(name="stp", bufs=2, space="PSUM"))
            psY = bctx.enter_context(tc.tile_pool(name="psY", bufs=1, space="PSUM"))
            bcp = bctx.enter_context(tc.tile_pool(name="bcp", bufs=1, space="PSUM"))
            exp_pool = bctx.enter_context(tc.tile_pool(name="expp", bufs=3))
            yc_pool = bctx.enter_context(tc.tile_pool(name="ycp", bufs=2))
            r_pool = bctx.enter_context(tc.tile_pool(name="rp", bufs=2))
            ynb_pool = bctx.enter_context(tc.tile_pool(name="ynbp", bufs=2))

            for p in range(NPAIR):
                qa = QK_sb[2 * p]
                ka = QK_sb[2 * p + 1]
                hA, hB = 2 * p, 2 * p + 1
                for qt in range(NT5):
                    nkt = 4 * qt + 4
                    q0 = qt * 512
                    pyA = psY.tile([65, 512], F32, name="pyA")
                    pyB = psY.tile([65, 512], F32, name="pyB")
                    pend = []  # software-pipeline AV one step behind QK on PE

                    def emit_av(kt, eAB, off, nkt=nkt, pyA=pyA, pyB=pyB,
                                hA=hA, hB=hB):
                        vt = V_sb[kt]
                        nc.tensor.matmul(
                            pyA[:, off:512], vt[:, hA * 65:(hA + 1) * 65],
                            eAB[:, off:512],
                            start=(kt == 0), stop=(kt == nkt - 1))
                        nc.tensor.matmul(
                            pyB[:, off:512], vt[:, hB * 65:(hB + 1) * 65],
                            eAB[:, 512 + off:1024],
                            start=(kt == 0), stop=(kt == nkt - 1))

                    for kt in range(nkt):
                        di = kt - 4 * qt
                        off = P * di if di > 0 else 0
                        L = 512 - off
                        st = st_pool.tile([P, 1024], F32, name="stAB")
                        nc.tensor.matmul(
                            st[:, off:512], ka[0:64, kt * P:(kt + 1) * P],
                            qa[0:64, q0 + off:q0 + 512])
                        nc.tensor.matmul(
                            st[:, 512 + off:1024], ka[64:128, kt * P:(kt + 1) * P],
                            qa[64:128, q0 + off:q0 + 512], tile_position=(64, 0))
                        st3 = st.rearrange("p (two n) -> p two n", two=2)[:, :, off:512]
                        if di >= 0:
                            mk = masks[di][:, None, off:512].broadcast_to([P, 2, L])
                            nc.vector.tensor_tensor(st3, st3, mk, op=ADD)
                        eAB = exp_pool.tile([P, 1024], BF16, name="eAB")
                        e3 = eAB.rearrange("p (two n) -> p two n", two=2)[:, :, off:512]
                        nc.scalar.activation(e3, st3, EXP, scale=SCALE)
                        pend.append((kt, eAB, off))
                        if len(pend) > 1:
                            emit_av(*pend.pop(0))
                    emit_av(*pend.pop(0))

                    # normalization
                    rrA = r_pool.tile([1, 512], F32, name="rrA")
                    nc.scalar.copy(rrA[:], pyA[64:65, :])
                    rrB = r_pool.tile([1, 512], F32, name="rrB")
                    nc.scalar.copy(rrB[:], pyB[64:65, :])
                    rA = r_pool.tile([1, 512], F32, name="rA")
                    rB = r_pool.tile([1, 512], F32, name="rB")
                    nc.vector.reciprocal_approx_fast(rA[:], rrA[:])
                    nc.vector.reciprocal_approx_fast(rB[:], rrB[:])
                    ycA = yc_pool.tile([64, 512], F32, name="ycA")
                    nc.vector.tensor_copy(ycA[:], pyA[0:64, :])
                    ycB = yc_pool.tile([64, 512], F32, name="ycB")
                    nc.vector.tensor_copy(ycB[:], pyB[0:64, :])
                    bcA = bcp.tile([64, 512], F32, name="bcA")
                    nc.tensor.matmul(bcA[:], ones[:], rA[:])
                    bcB = bcp.tile([64, 512], F32, name="bcB")
                    nc.tensor.matmul(bcB[:], ones[:], rB[:])
                    nc.vector.tensor_tensor(
                        Yn_sb[p][0:64, q0:q0 + 512], ycA[:], bcA[:], op=MULT)
                    ynB = ynb_pool.tile([64, 512], BF16, name="ynB")
                    nc.vector.tensor_tensor(ynB[:], ycB[:], bcB[:], op=MULT)
                    nc.sync.dma_start(out=Yn_sb[p][64:128, q0:q0 + 512], in_=ynB[:])

        # ---- stage C: out projection + chunked pair ReduceScatter ------
        with ExitStack() as cctx:
            wpr_pool = cctx.enter_context(tc.tile_pool(name="wprp", bufs=1))
            wpr_sb = [wpr_pool.tile([P, C], BF16, name=f"wpr{p}") for p in range(NPAIR)]
            for p in range(NPAIR):
                nc.sync.dma_start(out=wpr_sb[p][:], in_=wpr[p * P:(p + 1) * P, :])
            psO = cctx.enter_context(tc.tile_pool(name="psO", bufs=2, space="PSUM"))
            oc_pool = cctx.enter_context(tc.tile_pool(name="ocp", bufs=3))

            HROWS = T // (2 * NT5)  # per-core rows of one RS chunk
            for ts in range(NTS):
                for co in range(NCO):
                    po = psO.tile([P, 512], F32, name="po")
                    for p in range(NPAIR):
                        nc.tensor.matmul(
                            po[:], Yn_sb[p][:, ts * P:(ts + 1) * P],
                            wpr_sb[p][:, co * 512:(co + 1) * 512],
                            start=(p == 0), stop=(p == NPAIR - 1))
                    oc = oc_pool.tile([P, 512], F32, name="oc")
                    nc.vector.tensor_tensor(
                        oc[:], po[:], bprb_sb[:, co * 512:(co + 1) * 512], op=ADD)
                    nc.sync.dma_start(
                        out=ob[ts * P:(ts + 1) * P, co * 512:(co + 1) * 512],
                        in_=oc[:])
                if ts % 4 == 3:
                    c = ts // 4
                    nc.gpsimd.collective_compute(
                        "ReduceScatter", mybir.AluOpType.add,
                        replica_groups=groups,
                        ins=[ob[c * 512:(c + 1) * 512, :].opt()],
                        outs=[orh[c].opt()])
                    nc.sync.dma_start(
                        out=out[c * HROWS:(c + 1) * HROWS, :], in_=orh[c][:])

    nc.compile()
    return nc


def prep_core_inputs(x, w_qkv, b_qkv, w_proj, b_proj, b, g, HC):
    """Host-side shard + layout permutation for core (batch b, head group g)."""
    C = x.shape[-1]
    heads = [g * HC + i for i in range(HC)]
    wq, wk, wvf = w_qkv[:, 0:C], w_qkv[:, C:2 * C], w_qkv[:, 2 * C:3 * C]
    bq, bk, bvf = b_qkv[0:C], b_qkv[C:2 * C], b_qkv[2 * C:3 * C]
    qk_cols, bqk_cols = [], []
    for p in range(HC // 2):
        h0, h1 = heads[2 * p], heads[2 * p + 1]
        qk_cols += [wq[:, h0 * D:(h0 + 1) * D], wq[:, h1 * D:(h1 + 1) * D]]
        bqk_cols += [bq[h0 * D:(h0 + 1) * D], bq[h1 * D:(h1 + 1) * D]]
        qk_cols += [wk[:, h0 * D:(h0 + 1) * D], wk[:, h1 * D:(h1 + 1) * D]]
        bqk_cols += [bk[h0 * D:(h0 + 1) * D], bk[h1 * D:(h1 + 1) * D]]
    wqk = np.concatenate(qk_cols, axis=1).astype(BF16NP)
    bqk = np.concatenate(bqk_cols).astype(np.float32)
    bqk = np.ascontiguousarray(bqk.reshape(-1, P).T)
    wv = np.concatenate(
        [wvf[:, h * D:(h + 1) * D] for h in heads], axis=1).astype(BF16NP)
    bv = np.concatenate([bvf[h * D:(h + 1) * D] for h in heads]).astype(np.float32)
    bvb = np.ascontiguousarray(np.broadcast_to(bv[None, :], (P, bv.size)))
    wpr = np.concatenate(
        [w_proj[h * D:(h + 1) * D, :] for h in heads], axis=0).astype(BF16NP)
    bpr = (b_proj / 2.0).astype(np.float32)
    bprb = np.ascontiguousarray(np.broadcast_to(bpr[None, :], (P, C)))
    return {
        "xb": np.ascontiguousarray(x[b], np.float32),
        "wqk": np.ascontiguousarray(wqk), "bqk": bqk,
        "wv": np.ascontiguousarray(wv), "bvb": bvb,
        "wpr": np.ascontiguousarray(wpr), "bprb": bprb,
        "onespp": np.ones((P, P), BF16NP),
    }


def assemble_output(results, B, T, C):
    """Interleave the per-core ReduceScatter halves back to [B, T, C]."""
    NT5 = T // 512
    HROWS = T // (2 * NT5)
    full = np.empty((B, T, C), np.float32)
    for b in range(B):
        for r in range(2):
            o = np.asarray(results[2 * b + r]["out"])
            for c in range(NT5):
                dst = c * 512 + r * HROWS
                full[b, dst:dst + HROWS] = o[c * HROWS:(c + 1) * HROWS]
    return full


_CACHE = {}


def kernel(x, w_qkv, b_qkv, w_proj, b_proj, _trace=False):
    x = np.asarray(x, np.float32)
    w_qkv = np.asarray(w_qkv, np.float32)
    b_qkv = np.asarray(b_qkv, np.float32)
    w_proj = np.asarray(w_proj, np.float32)
    b_proj = np.asarray(b_proj, np.float32)
    B, T, C = x.shape
    H = 16
    HC = H // 2
    groups = [[2 * b, 2 * b + 1] for b in range(B)]

    key = (T, C, HC, 2 * B)
    if key not in _CACHE:
        _CACHE[key] = build_program(T, C, HC, 2 * B, groups)
    nc = _CACHE[key]

    in_maps = []
    for c in range(2 * B):
        in_maps.append(
            prep_core_inputs(x, w_qkv, b_qkv, w_proj, b_proj, c // 2, c % 2, HC))
    res = run_bass_kernel_spmd(nc, in_maps, list(range(2 * B)), trace=_trace)
    full = assemble_output(res.results, B, T, C)
    if _trace:
        return full, res
    return full


# revision 13
# speedup vs baseline: 2.0573x; 1.3254x over previous
"""Causal self-attention block (qkv proj -> causal MHA -> out proj) on 8 TRN2
NeuronCores.

Sharding: core c -> (batch b = c//2, head-group g = c%2). Each core computes
its batch's attention for 8 of the 16 heads (Megatron column-parallel qkv,
row-parallel out-proj), then a chunked pairwise ReduceScatter; each core
returns one half of its batch's rows and the host reassembles.

Pipeline structure (single TileContext, fully static):
  for qt in 0..3:
    attention for all head pairs on query tile qt (ACT-exp paced), with the
    NEXT t-tile's qkv-projection matmul work interleaved between attention
    slots to keep the PE dense; then the out-projection + ReduceScatter for
    the finished row chunk, which overlaps the next iteration.

Kernel-level choices:
  - Q^T/K^T produced directly in [c', t] orientation (w_qkv chunks stationary,
    PE-transposed x^T moving); head pairs packed 64+64 into 128-partition
    tiles so both heads' QK^T matmuls run concurrently via PE row tiling,
    writing two halves of one 2-bank PSUM tile; mask-add and exp are then one
    instruction per slot over both heads.
  - Diagonal blocks are column-trimmed to the causally-visible range.
  - V carries an all-ones column per head so the A@V matmul (M=65) emits
    softmax row-sums for free; normalization uses a fast approximate
    reciprocal and a partition-broadcast done by DMA (no PE/ACT cost).
  - All matmul operands are bf16 (fp32 accumulation in PSUM).
"""

import numpy as np
import ml_dtypes

import concourse.bass as bass
import concourse.bacc as bacc
import concourse.mybir as mybir
import concourse.tile as tile
from concourse.bass_utils import run_bass_kernel_spmd
from concourse.masks import make_identity
from contextlib import ExitStack

F32 = mybir.dt.float32
BF16 = mybir.dt.bfloat16
P = 128
D = 64
BF16NP = ml_dtypes.bfloat16


def build_program(T, C, HC, num_devices, groups):
    NPAIR = HC // 2
    CT = C // P            # contraction chunks of the qkv matmul
    NT5 = T // 512         # 512-wide t tiles (also RS chunks)
    NTS = T // P           # 128-wide t subtiles
    CSH = HC * D           # local c-tilde width (V / Y^T columns)
    NCO = C // 512         # 512-wide out-column tiles
    QKW = 2 * HC * D       # q+k c' columns
    NQK = QKW // P         # qk c'-tiles (2 per head pair)
    SCALE = 1.0 / np.sqrt(np.float32(D))
    NEG = -1.0e9
    HROWS = T // (2 * NT5)  # per-core rows of one RS chunk

    nc = bacc.Bacc("TRN2", target_bir_lowering=False, debug=False,
                   num_devices=num_devices)
    xb = nc.dram_tensor("xb", [T, C], F32, kind="ExternalInput").ap()
    wqk = nc.dram_tensor("wqk", [C, QKW], BF16, kind="ExternalInput").ap()
    wv = nc.dram_tensor("wv", [C, CSH], BF16, kind="ExternalInput").ap()
    bqk = nc.dram_tensor("bqk", [P, NQK], F32, kind="ExternalInput").ap()
    bvb = nc.dram_tensor("bvb", [P, CSH], F32, kind="ExternalInput").ap()
    bprb = nc.dram_tensor("bprb", [P, C], F32, kind="ExternalInput").ap()
    wpr = nc.dram_tensor("wpr", [CSH, C], BF16, kind="ExternalInput").ap()
    out = nc.dram_tensor("out", [T // 2, C], F32, kind="ExternalOutput").ap()

    ADD = mybir.AluOpType.add
    MULT = mybir.AluOpType.mult
    EXP = mybir.ActivationFunctionType.Exp

    with tile.TileContext(nc) as tc, ExitStack() as ctx:
        # ---- pools -----------------------------------------------------
        const = ctx.enter_context(tc.tile_pool(name="const", bufs=1))
        ident = const.tile([P, P], F32, name="ident")
        make_identity(nc, ident)
        bqk_sb = const.tile([P, NQK], F32, name="bqk_sb")
        nc.sync.dma_start(out=bqk_sb[:], in_=bqk[:])
        bvb_sb = const.tile([P, CSH], F32, name="bvb_sb")
        nc.sync.dma_start(out=bvb_sb[:], in_=bvb[:])
        bprb_sb = const.tile([P, C], F32, name="bprb_sb")
        nc.sync.dma_start(out=bprb_sb[:], in_=bprb[:])
        ones_bf = const.tile([1, 64], BF16, name="ones_bf")
        nc.vector.memset(ones_bf[:], 1.0)
        masks = []
        for i in range(4):
            mk = const.tile([P, 512], F32, name=f"mask{i}")
            nc.gpsimd.memset(mk[:], 0.0)
            nc.gpsimd.affine_select(
                out=mk[:], in_=mk[:], compare_op=mybir.AluOpType.is_ge,
                fill=NEG, base=-(P * i), pattern=[[1, 512]],
                channel_multiplier=-1)
            masks.append(mk)

        qk_pool = ctx.enter_context(tc.tile_pool(name="qkp", bufs=1))
        QK_sb = [qk_pool.tile([P, T], BF16, name=f"qk{ct}") for ct in range(NQK)]
        v_pool = ctx.enter_context(tc.tile_pool(name="vp", bufs=1))
        V_sb = [v_pool.tile([P, HC * 65], BF16, name=f"v{i}") for i in range(NTS)]
        yn_pool = ctx.enter_context(tc.tile_pool(name="ynp", bufs=1))
        Yn_sb = [yn_pool.tile([P, T], BF16, name=f"yn{p}") for p in range(NPAIR)]
        wpr_pool = ctx.enter_context(tc.tile_pool(name="wprp", bufs=1))
        wpr_sb = [wpr_pool.tile([P, C], BF16, name=f"wpr{p}") for p in range(NPAIR)]
        for p in range(NPAIR):
            nc.sync.dma_start(out=wpr_sb[p][:], in_=wpr[p * P:(p + 1) * P, :])
        wqk_pool = ctx.enter_context(tc.tile_pool(name="wqkp", bufs=1))
        wqk_sb = [wqk_pool.tile([P, QKW], BF16, name=f"wqk{j}") for j in range(CT)]
        for j in range(CT):
            nc.sync.dma_start(out=wqk_sb[j][:], in_=wqk[j * P:(j + 1) * P, :])
        wv_pool = ctx.enter_context(tc.tile_pool(name="wvp", bufs=1))
        wv_sb = [wv_pool.tile([P, CSH], BF16, name=f"wv{j}") for j in range(CT)]
        for j in range(CT):
            nc.sync.dma_start(out=wv_sb[j][:], in_=wv[j * P:(j + 1) * P, :])

        dram = ctx.enter_context(tc.tile_pool(name="dram", bufs=1, space="DRAM"))
        ob = [dram.tile([512, C], F32, name=f"ob{c}") for c in range(NT5)]
        orh = [dram.tile([HROWS, C], F32, name=f"orh{c}") for c in range(NT5)]

        xr_pool = ctx.enter_context(tc.tile_pool(name="xrp", bufs=8))
        xt_pool = ctx.enter_context(tc.tile_pool(name="xtp", bufs=2 * CT))
        exp_pool = ctx.enter_context(tc.tile_pool(name="expp", bufs=4))
        yc_pool = ctx.enter_context(tc.tile_pool(name="ycp", bufs=2))
        r_pool = ctx.enter_context(tc.tile_pool(name="rp", bufs=6))
        ynb_pool = ctx.enter_context(tc.tile_pool(name="ynbp", bufs=2))
        oc_pool = ctx.enter_context(tc.tile_pool(name="ocp", bufs=4))

        big = ctx.enter_context(tc.tile_pool(name="big", bufs=2, space="PSUM"))
        ps = ctx.enter_context(tc.tile_pool(name="ps", bufs=2, space="PSUM"))
        psY = ctx.enter_context(tc.tile_pool(name="psY", bufs=1, space="PSUM"))

        # ---- stage A emission units (qkv projection for t-tile tt) -----
        def stage_a_units(tt):
            units = []
            xrs = []
            xts = []

            def load_xr():
                for i in range(4):
                    xr = xr_pool.tile([P, C], F32, name="xr")
                    nc.sync.dma_start(
                        out=xr[:], in_=xb[(tt * 4 + i) * P:(tt * 4 + i + 1) * P, :])
                    xrs.append(xr)
            units.append(load_xr)

            def transpose(j):
                p_ = ps.tile([P, 512], F32, name="ps", tag="ps")
                for i in range(4):
                    nc.tensor.transpose(
                        p_[:, i * P:(i + 1) * P],
                        xrs[i][:, j * P:(j + 1) * P], ident[:])
                xt = xt_pool.tile([P, 512], BF16, name="xt")
                nc.scalar.copy(xt[:], p_[:])
                xts.append(xt)
            for j in range(CT):
                units.append(lambda j=j: transpose(j))

            def qkt(ct):
                p_ = ps.tile([P, 512], F32, name="ps", tag="ps")
                for j in range(CT):
                    nc.tensor.matmul(
                        p_[:], wqk_sb[j][:, ct * P:(ct + 1) * P],
                        xts[j][:], start=(j == 0), stop=(j == CT - 1))
                nc.vector.tensor_scalar_add(
                    QK_sb[ct][:, tt * 512:(tt + 1) * 512], p_[:],
                    bqk_sb[:, ct:ct + 1])
            for ct in range(NQK):
                units.append(lambda ct=ct: qkt(ct))

            def vproj(i):
                p_ = ps.tile([P, CSH], F32, name="ps", tag="ps")
                for j in range(CT):
                    nc.tensor.matmul(
                        p_[:], xts[j][:, i * P:(i + 1) * P],
                        wv_sb[j][:], start=(j == 0), stop=(j == CT - 1))
                vt = V_sb[tt * 4 + i]
                vt3 = vt.rearrange("p (h e) -> p h e", e=65)
                nc.vector.tensor_tensor(
                    vt3[:, :, 0:64], p_.rearrange("p (h d) -> p h d", d=D),
                    bvb_sb.rearrange("p (h d) -> p h d", d=D), op=ADD)
                nc.vector.memset(vt3[:, :, 64:65], 1.0)
            for i in range(4):
                units.append(lambda i=i: vproj(i))
            return units

        # ---- stage B: attention for query tile qt, one head pair -------
        def attention(p, qt, a_units):
            qa = QK_sb[2 * p]
            ka = QK_sb[2 * p + 1]
            hA, hB = 2 * p, 2 * p + 1
            nkt = 4 * qt + 4
            q0 = qt * 512
            pyA = psY.tile([65, 512], F32, name="pyA")
            pyB = psY.tile([65, 512], F32, name="pyB")
            pend = []  # software-pipeline AV one step behind QK on PE

            def emit_av(kt, eAB, off):
                vt = V_sb[kt]
                nc.tensor.matmul(
                    pyA[:, off:512], vt[:, hA * 65:(hA + 1) * 65],
                    eAB[:, off:512],
                    start=(kt == 0), stop=(kt == nkt - 1))
                nc.tensor.matmul(
                    pyB[:, off:512], vt[:, hB * 65:(hB + 1) * 65],
                    eAB[:, 512 + off:1024],
                    start=(kt == 0), stop=(kt == nkt - 1))

            for kt in range(nkt):
                di = kt - 4 * qt
                off = P * di if di > 0 else 0
                L = 512 - off
                st = big.tile([P, 1024], F32, name="stAB")
                nc.tensor.matmul(
                    st[:, off:512], ka[0:64, kt * P:(kt + 1) * P],
                    qa[0:64, q0 + off:q0 + 512])
                nc.tensor.matmul(
                    st[:, 512 + off:1024], ka[64:128, kt * P:(kt + 1) * P],
                    qa[64:128, q0 + off:q0 + 512], tile_position=(64, 0))
                st3 = st.rearrange("p (two n) -> p two n", two=2)[:, :, off:512]
                if di >= 0:
                    mk = masks[di][:, None, off:512].broadcast_to([P, 2, L])
                    nc.vector.tensor_tensor(st3, st3, mk, op=ADD)
                eAB = exp_pool.tile([P, 1024], BF16, name="eAB")
                e3 = eAB.rearrange("p (two n) -> p two n", two=2)[:, :, off:512]
                nc.scalar.activation(e3, st3, EXP, scale=SCALE)
                pend.append((kt, eAB, off))
                if len(pend) > 1:
                    emit_av(*pend.pop(0))
                if a_units:
                    a_units.pop(0)()
            emit_av(*pend.pop(0))

            # normalization: Y[:, q] /= rowsum[q]
            rrA = r_pool.tile([1, 512], F32, name="rrA", tag="rw")
            nc.scalar.copy(rrA[:], pyA[64:65, :])
            rrB = r_pool.tile([1, 512], F32, name="rrB", tag="rw")
            nc.scalar.copy(rrB[:], pyB[64:65, :])
            ycA = yc_pool.tile([64, 512], F32, name="ycA")
            nc.vector.tensor_copy(ycA[:], pyA[0:64, :])
            ycB = yc_pool.tile([64, 512], F32, name="ycB")
            nc.vector.tensor_copy(ycB[:], pyB[0:64, :])
            rA = r_pool.tile([1, 512], F32, name="rA", tag="rw")
            nc.vector.reciprocal_approx_fast(rA[:], rrA[:])
            rB = r_pool.tile([1, 512], F32, name="rB", tag="rw")
            nc.vector.reciprocal_approx_fast(rB[:], rrB[:])
            rAb = r_pool.tile([1, 512], BF16, name="rAb", tag="rw")
            nc.vector.tensor_copy(rAb[:], rA[:])
            rBb = r_pool.tile([1, 512], BF16, name="rBb", tag="rw")
            nc.vector.tensor_copy(rBb[:], rB[:])
            bcA = ps.tile([64, 512], F32, name="ps", tag="ps")
            nc.tensor.matmul(bcA[:], ones_bf[:], rAb[:])
            bcB = ps.tile([64, 512], F32, name="ps", tag="ps")
            nc.tensor.matmul(bcB[:], ones_bf[:], rBb[:])
            nc.vector.tensor_tensor(
                Yn_sb[p][0:64, q0:q0 + 512], ycA[:], bcA[:], op=MULT)
            ynB = ynb_pool.tile([64, 512], BF16, name="ynB")
            nc.vector.tensor_tensor(ynB[:], ycB[:], bcB[:], op=MULT)
            nc.sync.dma_start(out=Yn_sb[p][64:128, q0:q0 + 512], in_=ynB[:])

        # ---- stage C: out projection + ReduceScatter for chunk qt ------
        def proj_chunk(qt):
            for ts in range(4 * qt, 4 * qt + 4):
                for co in range(NCO):
                    po = ps.tile([P, 512], F32, name="ps", tag="ps")
                    for p in range(NPAIR):
                        nc.tensor.matmul(
                            po[:], Yn_sb[p][:, ts * P:(ts + 1) * P],
                            wpr_sb[p][:, co * 512:(co + 1) * 512],
                            start=(p == 0), stop=(p == NPAIR - 1))
                    oc = oc_pool.tile([P, 512], F32, name="oc")
                    nc.vector.tensor_tensor(
                        oc[:], po[:], bprb_sb[:, co * 512:(co + 1) * 512], op=ADD)
                    nc.sync.dma_start(
                        out=ob[qt][(ts - 4 * qt) * P:(ts - 4 * qt + 1) * P,
                                   co * 512:(co + 1) * 512],
                        in_=oc[:])
            nc.gpsimd.collective_compute(
                "ReduceScatter", mybir.AluOpType.add, replica_groups=groups,
                ins=[ob[qt].opt()], outs=[orh[qt].opt()])
            nc.sync.dma_start(
                out=out[qt * HROWS:(qt + 1) * HROWS, :], in_=orh[qt][:])

        # ---- top-level pipeline ---------------------------------------
        a_units = stage_a_units(0)
        for u in a_units:
            u()
        a_units = stage_a_units(1) if NT5 > 1 else []
        for qt in range(NT5):
            for p in range(NPAIR):
                attention(p, qt, a_units)
            for u in a_units:  # flush leftovers before they're needed
                u()
            a_units = stage_a_units(qt + 2) if qt + 2 < NT5 else []
            proj_chunk(qt)

    nc.compile()
    return nc


def prep_core_inputs(x, w_qkv, b_qkv, w_proj, b_proj, b, g, HC):
    """Host-side shard + layout permutation for core (batch b, head group g)."""
    C = x.shape[-1]
    heads = [g * HC + i for i in range(HC)]
    wq, wk, wvf = w_qkv[:, 0:C], w_qkv[:, C:2 * C], w_qkv[:, 2 * C:3 * C]
    bq, bk, bvf = b_qkv[0:C], b_qkv[C:2 * C], b_qkv[2 * C:3 * C]
    qk_cols, bqk_cols = [], []
    for p in range(HC // 2):
        h0, h1 = heads[2 * p], heads[2 * p + 1]
        qk_cols += [wq[:, h0 * D:(h0 + 1) * D], wq[:, h1 * D:(h1 + 1) * D]]
        bqk_cols += [bq[h0 * D:(h0 + 1) * D], bq[h1 * D:(h1 + 1) * D]]
        qk_cols += [wk[:, h0 * D:(h0 + 1) * D], wk[:, h1 * D:(h1 + 1) * D]]
        bqk_cols += [bk[h0 * D:(h0 + 1) * D], bk[h1 * D:(h1 + 1) * D]]
    wqk = np.concatenate(qk_cols, axis=1).astype(BF16NP)
    bqk = np.concatenate(bqk_cols).astype(np.float32)
    bqk = np.ascontiguousarray(bqk.reshape(-1, P).T)
    wv = np.concatenate(
        [wvf[:, h * D:(h + 1) * D] for h in heads], axis=1).astype(BF16NP)
    bv = np.concatenate([bvf[h * D:(h + 1) * D] for h in heads]).astype(np.float32)
    bvb = np.ascontiguousarray(np.broadcast_to(bv[None, :], (P, bv.size)))
    wpr = np.concatenate(
        [w_proj[h * D:(h + 1) * D, :] for h in heads], axis=0).astype(BF16NP)
    bpr = (b_proj / 2.0).astype(np.float32)
    bprb = np.ascontiguousarray(np.broadcast_to(bpr[None, :], (P, C)))
    return {
        "xb": np.ascontiguousarray(x[b], np.float32),
        "wqk": np.ascontiguousarray(wqk), "bqk": bqk,
        "wv": np.ascontiguousarray(wv), "bvb": bvb,
        "wpr": np.ascontiguousarray(wpr), "bprb": bprb,
    }


def assemble_output(results, B, T, C):
    """Interleave the per-core ReduceScatter halves back to [B, T, C]."""
    NT5 = T // 512
    HROWS = T // (2 * NT5)
    full = np.empty((B, T, C), np.float32)
    for b in range(B):
        for r in range(2):
            o = np.asarray(results[2 * b + r]["out"])
            for c in range(NT5):
                dst = c * 512 + r * HROWS
                full[b, dst:dst + HROWS] = o[c * HROWS:(c + 1) * HROWS]
    return full


_CACHE = {}


def kernel(x, w_qkv, b_qkv, w_proj, b_proj, _trace=False):
    x = np.asarray(x, np.float32)
    w_qkv = np.asarray(w_qkv, np.float32)
    b_qkv = np.asarray(b_qkv, np.float32)
    w_proj = np.asarray(w_proj, np.float32)
    b_proj = np.asarray(b_proj, np.float32)
    B, T, C = x.shape
    H = 16
    HC = H // 2
    groups = [[2 * b, 2 * b + 1] for b in range(B)]

    key = (T, C, HC, 2 * B)
    if key not in _CACHE:
        _CACHE[key] = build_program(T, C, HC, 2 * B, groups)
    nc = _CACHE[key]

    in_maps = []
    for c in range(2 * B):
        in_maps.append(
            prep_core_inputs(x, w_qkv, b_qkv, w_proj, b_proj, c // 2, c % 2, HC))
    res = run_bass_kernel_spmd(nc, in_maps, list(range(2 * B)), trace=_trace)
    full = assemble_output(res.results, B, T, C)
    if _trace:
        return full, res
    return full


# revision 14
# speedup vs baseline: 2.1548x; 1.0474x over previous
"""Causal self-attention block (qkv proj -> causal MHA -> out proj) on 8 TRN2
NeuronCores.

Sharding: core c -> (batch b = c//2, head-group g = c%2). Each core computes
its batch's attention for 8 of the 16 heads (Megatron column-parallel qkv,
row-parallel out-proj), then a chunked pairwise ReduceScatter; each core
returns one half of its batch's rows and the host reassembles.

Pipeline structure (single TileContext, fully static):
  for qt in 0..3:
    attention for all head pairs on query tile qt (ACT-exp paced), with the
    NEXT t-tile's qkv-projection matmul work interleaved between attention
    slots to keep the PE dense; then the out-projection + ReduceScatter for
    the finished row chunk, which overlaps the next iteration.

Kernel-level choices:
  - Q^T/K^T produced directly in [c', t] orientation (w_qkv chunks stationary,
    PE-transposed x^T moving); head pairs packed 64+64 into 128-partition
    tiles so both heads' QK^T matmuls run concurrently via PE row tiling,
    writing two halves of one 2-bank PSUM tile; mask-add and exp are then one
    instruction per slot over both heads.
  - Diagonal blocks are column-trimmed to the causally-visible range.
  - V carries an all-ones column per head so the A@V matmul (M=65) emits
    softmax row-sums for free; normalization uses a fast approximate
    reciprocal and a partition-broadcast done by DMA (no PE/ACT cost).
  - All matmul operands are bf16 (fp32 accumulation in PSUM).
"""

import numpy as np
import ml_dtypes

import concourse.bass as bass
import concourse.bacc as bacc
import concourse.mybir as mybir
import concourse.tile as tile
from concourse.bass_utils import run_bass_kernel_spmd
from concourse.masks import make_identity
from contextlib import ExitStack

F32 = mybir.dt.float32
BF16 = mybir.dt.bfloat16
P = 128
D = 64
BF16NP = ml_dtypes.bfloat16


def build_program(T, C, HC, num_devices, groups):
    NPAIR = HC // 2
    CT = C // P            # contraction chunks of the qkv matmul
    NT5 = T // 512         # 512-wide t tiles (also RS chunks)
    NTS = T // P           # 128-wide t subtiles
    CSH = HC * D           # local c-tilde width (V / Y^T columns)
    NCO = C // 512         # 512-wide out-column tiles
    QKW = 2 * HC * D       # q+k c' columns
    NQK = QKW // P         # qk c'-tiles (2 per head pair)
    SCALE = 1.0 / np.sqrt(np.float32(D))
    NEG = -1.0e9
    HROWS = T // (2 * NT5)  # per-core rows of one RS chunk

    nc = bacc.Bacc("TRN2", target_bir_lowering=False, debug=False,
                   num_devices=num_devices)
    xbf = nc.dram_tensor("xbf", [T, C], BF16, kind="ExternalInput").ap()
    wqk = nc.dram_tensor("wqk", [C, QKW], BF16, kind="ExternalInput").ap()
    wv = nc.dram_tensor("wv", [C, CSH], BF16, kind="ExternalInput").ap()
    bqk = nc.dram_tensor("bqk", [P, NQK], F32, kind="ExternalInput").ap()
    bvb = nc.dram_tensor("bvb", [P, CSH], F32, kind="ExternalInput").ap()
    bprb = nc.dram_tensor("bprb", [P, C], F32, kind="ExternalInput").ap()
    wpr = nc.dram_tensor("wpr", [CSH, C], BF16, kind="ExternalInput").ap()
    out = nc.dram_tensor("out", [T // 2, C], F32, kind="ExternalOutput").ap()

    ADD = mybir.AluOpType.add
    MULT = mybir.AluOpType.mult
    EXP = mybir.ActivationFunctionType.Exp

    with tile.TileContext(nc) as tc, ExitStack() as ctx:
        # ---- pools -----------------------------------------------------
        const = ctx.enter_context(tc.tile_pool(name="const", bufs=1))
        bqk_sb = const.tile([P, NQK], F32, name="bqk_sb")
        nc.sync.dma_start(out=bqk_sb[:], in_=bqk[:])
        bvb_sb = const.tile([P, CSH], F32, name="bvb_sb")
        nc.sync.dma_start(out=bvb_sb[:], in_=bvb[:])
        bprb_sb = const.tile([P, C], F32, name="bprb_sb")
        nc.sync.dma_start(out=bprb_sb[:], in_=bprb[:])
        ones_bf = const.tile([1, 64], BF16, name="ones_bf")
        nc.vector.memset(ones_bf[:], 1.0)
        masks = []
        for i in range(4):
            mk = const.tile([P, 512], F32, name=f"mask{i}")
            nc.gpsimd.memset(mk[:], 0.0)
            nc.gpsimd.affine_select(
                out=mk[:], in_=mk[:], compare_op=mybir.AluOpType.is_ge,
                fill=NEG, base=-(P * i), pattern=[[1, 512]],
                channel_multiplier=-1)
            masks.append(mk)

        qk_pool = ctx.enter_context(tc.tile_pool(name="qkp", bufs=1))
        QK_sb = [qk_pool.tile([P, T], BF16, name=f"qk{ct}") for ct in range(NQK)]
        v_pool = ctx.enter_context(tc.tile_pool(name="vp", bufs=1))
        V_sb = [v_pool.tile([P, HC * 65], BF16, name=f"v{i}") for i in range(NTS)]
        yn_pool = ctx.enter_context(tc.tile_pool(name="ynp", bufs=1))
        Yn_sb = [yn_pool.tile([P, T], BF16, name=f"yn{p}") for p in range(NPAIR)]
        wpr_pool = ctx.enter_context(tc.tile_pool(name="wprp", bufs=1))
        wpr_sb = [wpr_pool.tile([P, C], BF16, name=f"wpr{p}") for p in range(NPAIR)]
        for p in range(NPAIR):
            nc.sync.dma_start(out=wpr_sb[p][:], in_=wpr[p * P:(p + 1) * P, :])
        wqk_pool = ctx.enter_context(tc.tile_pool(name="wqkp", bufs=1))
        wqk_sb = [wqk_pool.tile([P, QKW], BF16, name=f"wqk{j}") for j in range(CT)]
        for j in range(CT):
            nc.sync.dma_start(out=wqk_sb[j][:], in_=wqk[j * P:(j + 1) * P, :])
        wv_pool = ctx.enter_context(tc.tile_pool(name="wvp", bufs=1))
        wv_sb = [wv_pool.tile([P, CSH], BF16, name=f"wv{j}") for j in range(CT)]
        for j in range(CT):
            nc.sync.dma_start(out=wv_sb[j][:], in_=wv[j * P:(j + 1) * P, :])

        dram = ctx.enter_context(tc.tile_pool(name="dram", bufs=1, space="DRAM"))
        ob = [dram.tile([512, C], F32, name=f"ob{c}") for c in range(NT5)]
        orh = [dram.tile([HROWS, C], F32, name=f"orh{c}") for c in range(NT5)]

        xt_pool = ctx.enter_context(tc.tile_pool(name="xtp", bufs=2 * CT))
        exp_pool = ctx.enter_context(tc.tile_pool(name="expp", bufs=4))
        yc_pool = ctx.enter_context(tc.tile_pool(name="ycp", bufs=2))
        r_pool = ctx.enter_context(tc.tile_pool(name="rp", bufs=6))
        ynb_pool = ctx.enter_context(tc.tile_pool(name="ynbp", bufs=2))
        oc_pool = ctx.enter_context(tc.tile_pool(name="ocp", bufs=4))

        big = ctx.enter_context(tc.tile_pool(name="big", bufs=2, space="PSUM"))
        ps = ctx.enter_context(tc.tile_pool(name="ps", bufs=2, space="PSUM"))
        psY = ctx.enter_context(tc.tile_pool(name="psY", bufs=1, space="PSUM"))

        # ---- stage A emission units (qkv projection for t-tile tt) -----
        def stage_a_units(tt):
            units = []
            xts = []

            def load_xt():
                for j in range(CT):
                    xt = xt_pool.tile([P, 512], BF16, name="xt")
                    nc.sync.dma_start(
                        out=xt[:],
                        in_=xbf[tt * 512:(tt + 1) * 512, j * P:(j + 1) * P],
                        transpose=True)
                    xts.append(xt)
            units.append(load_xt)

            def qkt(ct):
                p_ = ps.tile([P, 512], F32, name="ps", tag="ps")
                for j in range(CT):
                    nc.tensor.matmul(
                        p_[:], wqk_sb[j][:, ct * P:(ct + 1) * P],
                        xts[j][:], start=(j == 0), stop=(j == CT - 1))
                nc.vector.tensor_scalar_add(
                    QK_sb[ct][:, tt * 512:(tt + 1) * 512], p_[:],
                    bqk_sb[:, ct:ct + 1])
            for ct in range(NQK):
                units.append(lambda ct=ct: qkt(ct))

            def vproj(i):
                p_ = ps.tile([P, CSH], F32, name="ps", tag="ps")
                for j in range(CT):
                    nc.tensor.matmul(
                        p_[:], xts[j][:, i * P:(i + 1) * P],
                        wv_sb[j][:], start=(j == 0), stop=(j == CT - 1))
                vt = V_sb[tt * 4 + i]
                vt3 = vt.rearrange("p (h e) -> p h e", e=65)
                nc.vector.tensor_tensor(
                    vt3[:, :, 0:64], p_.rearrange("p (h d) -> p h d", d=D),
                    bvb_sb.rearrange("p (h d) -> p h d", d=D), op=ADD)
                nc.vector.memset(vt3[:, :, 64:65], 1.0)
            for i in range(4):
                units.append(lambda i=i: vproj(i))
            return units

        # ---- stage B: attention for query tile qt, one head pair -------
        def attention(p, qt, a_units):
            qa = QK_sb[2 * p]
            ka = QK_sb[2 * p + 1]
            hA, hB = 2 * p, 2 * p + 1
            nkt = 4 * qt + 4
            q0 = qt * 512
            pyA = psY.tile([65, 512], F32, name="pyA")
            pyB = psY.tile([65, 512], F32, name="pyB")
            pend = []  # software-pipeline AV one step behind QK on PE

            def emit_av(kt, eAB, off):
                vt = V_sb[kt]
                nc.tensor.matmul(
                    pyA[:, off:512], vt[:, hA * 65:(hA + 1) * 65],
                    eAB[:, off:512],
                    start=(kt == 0), stop=(kt == nkt - 1))
                nc.tensor.matmul(
                    pyB[:, off:512], vt[:, hB * 65:(hB + 1) * 65],
                    eAB[:, 512 + off:1024],
                    start=(kt == 0), stop=(kt == nkt - 1))

            for kt in range(nkt):
                di = kt - 4 * qt
                off = P * di if di > 0 else 0
                L = 512 - off
                st = big.tile([P, 1024], F32, name="stAB")
                nc.tensor.matmul(
                    st[:, off:512], ka[0:64, kt * P:(kt + 1) * P],
                    qa[0:64, q0 + off:q0 + 512])
                nc.tensor.matmul(
                    st[:, 512 + off:1024], ka[64:128, kt * P:(kt + 1) * P],
                    qa[64:128, q0 + off:q0 + 512], tile_position=(64, 0))
                st3 = st.rearrange("p (two n) -> p two n", two=2)[:, :, off:512]
                if di >= 0:
                    mk = masks[di][:, None, off:512].broadcast_to([P, 2, L])
                    nc.vector.tensor_tensor(st3, st3, mk, op=ADD)
                eAB = exp_pool.tile([P, 1024], BF16, name="eAB")
                e3 = eAB.rearrange("p (two n) -> p two n", two=2)[:, :, off:512]
                nc.scalar.activation(e3, st3, EXP, scale=SCALE)
                pend.append((kt, eAB, off))
                if a_units:
                    a_units.pop(0)()
                if len(pend) > 2:
                    emit_av(*pend.pop(0))
            while pend:
                emit_av(*pend.pop(0))

            # normalization: Y[:, q] /= rowsum[q]
            rrA = r_pool.tile([1, 512], F32, name="rrA", tag="rw")
            nc.scalar.copy(rrA[:], pyA[64:65, :])
            rrB = r_pool.tile([1, 512], F32, name="rrB", tag="rw")
            nc.scalar.copy(rrB[:], pyB[64:65, :])
            ycA = yc_pool.tile([64, 512], F32, name="ycA")
            nc.vector.tensor_copy(ycA[:], pyA[0:64, :])
            ycB = yc_pool.tile([64, 512], F32, name="ycB")
            nc.vector.tensor_copy(ycB[:], pyB[0:64, :])
            rA = r_pool.tile([1, 512], F32, name="rA", tag="rw")
            nc.vector.reciprocal_approx_fast(rA[:], rrA[:])
            rB = r_pool.tile([1, 512], F32, name="rB", tag="rw")
            nc.vector.reciprocal_approx_fast(rB[:], rrB[:])
            rAb = r_pool.tile([1, 512], BF16, name="rAb", tag="rw")
            nc.vector.tensor_copy(rAb[:], rA[:])
            rBb = r_pool.tile([1, 512], BF16, name="rBb", tag="rw")
            nc.vector.tensor_copy(rBb[:], rB[:])
            bcA = ps.tile([64, 512], F32, name="ps", tag="ps")
            nc.tensor.matmul(bcA[:], ones_bf[:], rAb[:])
            bcB = ps.tile([64, 512], F32, name="ps", tag="ps")
            nc.tensor.matmul(bcB[:], ones_bf[:], rBb[:])
            nc.vector.tensor_tensor(
                Yn_sb[p][0:64, q0:q0 + 512], ycA[:], bcA[:], op=MULT)
            ynB = ynb_pool.tile([64, 512], BF16, name="ynB")
            nc.vector.tensor_tensor(ynB[:], ycB[:], bcB[:], op=MULT)
            nc.sync.dma_start(out=Yn_sb[p][64:128, q0:q0 + 512], in_=ynB[:])

        # ---- stage C: out projection + ReduceScatter for chunk qt ------
        def proj_units(qt):
            units = []

            def ts_unit(ts, co):
                po = ps.tile([P, 512], F32, name="ps", tag="ps")
                for p in range(NPAIR):
                    nc.tensor.matmul(
                        po[:], Yn_sb[p][:, ts * P:(ts + 1) * P],
                        wpr_sb[p][:, co * 512:(co + 1) * 512],
                        start=(p == 0), stop=(p == NPAIR - 1))
                oc = oc_pool.tile([P, 512], F32, name="oc")
                nc.vector.tensor_tensor(
                    oc[:], po[:], bprb_sb[:, co * 512:(co + 1) * 512], op=ADD)
                nc.sync.dma_start(
                    out=ob[qt][(ts - 4 * qt) * P:(ts - 4 * qt + 1) * P,
                               co * 512:(co + 1) * 512],
                    in_=oc[:])
            for ts in range(4 * qt, 4 * qt + 4):
                for co in range(NCO):
                    units.append(lambda ts=ts, co=co: ts_unit(ts, co))

            def rs_unit():
                nc.gpsimd.collective_compute(
                    "ReduceScatter", mybir.AluOpType.add, replica_groups=groups,
                    ins=[ob[qt].opt()], outs=[orh[qt].opt()])
                nc.sync.dma_start(
                    out=out[qt * HROWS:(qt + 1) * HROWS, :], in_=orh[qt][:])
            units.append(rs_unit)
            return units

        # ---- top-level pipeline ---------------------------------------
        # Process query tiles in order [1, 2, 3, 0]; fillers keep the PE
        # dense during the ACT-paced attention slots: the next t-tile's qkv
        # projection and the previous chunk's out-projection + RS.
        if NT5 == 4:
            for u in stage_a_units(0) + stage_a_units(1):
                u()
            fill = {1: stage_a_units(2),
                    2: stage_a_units(3) + proj_units(1),
                    3: proj_units(2),
                    0: proj_units(3)}
            order = [1, 2, 3, 0]
            for qt in order:
                fillers = fill[qt]
                for p in range(NPAIR):
                    attention(p, qt, fillers)
                for u in fillers:
                    u()
            for u in proj_units(0):
                u()
        else:  # small test configs: plain order
            fillers = []
            for u in stage_a_units(0):
                u()
            for qt in range(NT5):
                for u in fillers:
                    u()
                if qt + 1 < NT5:
                    for u in stage_a_units(qt + 1):
                        u()
                for p in range(NPAIR):
                    attention(p, qt, [])
                fillers = proj_units(qt)
            for u in fillers:
                u()

    nc.compile()
    return nc


def prep_core_inputs(x, w_qkv, b_qkv, w_proj, b_proj, b, g, HC):
    """Host-side shard + layout permutation for core (batch b, head group g)."""
    C = x.shape[-1]
    heads = [g * HC + i for i in range(HC)]
    wq, wk, wvf = w_qkv[:, 0:C], w_qkv[:, C:2 * C], w_qkv[:, 2 * C:3 * C]
    bq, bk, bvf = b_qkv[0:C], b_qkv[C:2 * C], b_qkv[2 * C:3 * C]
    qk_cols, bqk_cols = [], []
    for p in range(HC // 2):
        h0, h1 = heads[2 * p], heads[2 * p + 1]
        qk_cols += [wq[:, h0 * D:(h0 + 1) * D], wq[:, h1 * D:(h1 + 1) * D]]
        bqk_cols += [bq[h0 * D:(h0 + 1) * D], bq[h1 * D:(h1 + 1) * D]]
        qk_cols += [wk[:, h0 * D:(h0 + 1) * D], wk[:, h1 * D:(h1 + 1) * D]]
        bqk_cols += [bk[h0 * D:(h0 + 1) * D], bk[h1 * D:(h1 + 1) * D]]
    wqk = np.concatenate(qk_cols, axis=1).astype(BF16NP)
    bqk = np.concatenate(bqk_cols).astype(np.float32)
    bqk = np.ascontiguousarray(bqk.reshape(-1, P).T)
    wv = np.concatenate(
        [wvf[:, h * D:(h + 1) * D] for h in heads], axis=1).astype(BF16NP)
    bv = np.concatenate([bvf[h * D:(h + 1) * D] for h in heads]).astype(np.float32)
    bvb = np.ascontiguousarray(np.broadcast_to(bv[None, :], (P, bv.size)))
    wpr = np.concatenate(
        [w_proj[h * D:(h + 1) * D, :] for h in heads], axis=0).astype(BF16NP)
    bpr = (b_proj / 2.0).astype(np.float32)
    bprb = np.ascontiguousarray(np.broadcast_to(bpr[None, :], (P, C)))
    return {
        "xbf": np.ascontiguousarray(x[b].astype(BF16NP)),
        "wqk": np.ascontiguousarray(wqk), "bqk": bqk,
        "wv": np.ascontiguousarray(wv), "bvb": bvb,
        "wpr": np.ascontiguousarray(wpr), "bprb": bprb,
    }


def assemble_output(results, B, T, C):
    """Interleave the per-core ReduceScatter halves back to [B, T, C]."""
    NT5 = T // 512
    HROWS = T // (2 * NT5)
    full = np.empty((B, T, C), np.float32)
    for b in range(B):
        for r in range(2):
            o = np.asarray(results[2 * b + r]["out"])
            for c in range(NT5):
                dst = c * 512 + r * HROWS
                full[b, dst:dst + HROWS] = o[c * HROWS:(c + 1) * HROWS]
    return full


_CACHE = {}


def kernel(x, w_qkv, b_qkv, w_proj, b_proj, _trace=False):
    x = np.asarray(x, np.float32)
    w_qkv = np.asarray(w_qkv, np.float32)
    b_qkv = np.asarray(b_qkv, np.float32)
    w_proj = np.asarray(w_proj, np.float32)
    b_proj = np.asarray(b_proj, np.float32)
    B, T, C = x.shape
    H = 16
    HC = H // 2
    groups = [[2 * b, 2 * b + 1] for b in range(B)]

    key = (T, C, HC, 2 * B)
    if key not in _CACHE:
        _CACHE[key] = build_program(T, C, HC, 2 * B, groups)
    nc = _CACHE[key]

    in_maps = []
    for c in range(2 * B):
        in_maps.append(
            prep_core_inputs(x, w_qkv, b_qkv, w_proj, b_proj, c // 2, c % 2, HC))
    res = run_bass_kernel_spmd(nc, in_maps, list(range(2 * B)), trace=_trace)
    full = assemble_output(res.results, B, T, C)
    if _trace:
        return full, res
    return full
